# revision 29
# baseline (speedup 1.0000x reference)
"""Trainium2 Bass kernel for the two-template sparse cross-modal attention module.

Sharding: data-parallel over batch B=32 across 8 NeuronCores (4 samples/core).
Each sample carries two modality streams (v, i) that must be co-resident
because search tokens attend to the template keys of BOTH modalities.

Per-core program (per sample s, streams st in {v, i}), all matmuls bf16 with
fp32 PSUM accumulation (inputs are cast to bf16 by the gpsimd DMAs, which
also halves input HBM traffic):
  1. QK^T projection in transposed layout (QKT[1536, 384] = qkv_w[0:1536] @
     x.T, stored bf16); V projection in natural layout with a ones column per
     head ([tok, 65]) so the AV matmul also accumulates the softmax
     denominator.
  2. Scores transposed (S.T[k, q] = K Q.T, contract Dh=64), exp on the ACT
     engine (no max-subtraction; logits are O(1)). Each score matmul writes a
     whole single-bank PSUM tile at offset 0: a start=True accumulation-group
     open at a non-zero PSUM bank offset faults on real hardware.
  3. AV in natural orientation: out[q, 65] = es.T @ V1 with es (the exp'ed
     score tile) as the stationary operand — each matmul streams only 65
     columns instead of 256/384, which is what the PE cost scales with. The
     six accumulation chains of a head-pair share one PSUM bank as ONE group
     (only the offset-0 chain opens it). Normalization = DVE batched
     reciprocal of the ones column + per-head scaled PSUM-drain copies into
     the natural-layout O tile (bf16), emitted in reverse so the bank is
     fully drained before the next group's first AV write (PSUM banks are
     single-ported; a PE write concurrent with a DVE read anywhere in the
     bank is fatal).
  4. O is transposed back to channel-major via PE transpose instructions
     (bf16 identity), then the output projection runs from the transposed
     tile; the bias is added by the DVE during the PSUM drain (no K=1 bias
     matmuls).

Scheduling: projection matmul blocks are held in a keyed filler queue and
interleaved between each head-pair's score and AV matmuls so the ACT engine's
exp latency (the attention-phase bottleneck) hides under PE work. Each window
force-emits just-in-time the P1/P2 blocks the next window's scores/AV read
and the deferred output-projection blocks whose o_nat buffers are about to be
recycled; surplus blocks ride across sample boundaries toward the
filler-starved last sample. Input DMAs are issued as batched multi-chunk
waves in exact consumption order, so the first projection matmul starts ~6us
into the program.

Cost-model numbers (TimelineSim, per core): 288us total, PE busy 265us
(vs 412us / 290us for the previous kernel).
"""

import numpy as np

for _p in ("/opt/trn_rl_repo", "/root/.axon_site/_ro/trn_rl_repo"):
    import os
    import sys

    if os.path.isdir(_p) and _p not in sys.path:
        sys.path.append(_p)

B = 32
N_CORES = 8
SAMPLES = 4  # per core
C = 768
NTOK = 384
H = 12
DH = 64
MT = 128  # template tokens
CCH = C // 128  # 6 contraction chunks
MCH = 12  # QK row chunks (1536/128)
TCH = NTOK // 128  # 3 token chunks
SCALE = DH ** (-0.5)
# fp8 double-row projection path: inputs are scaled then split into
# e4m3 hi + e4m3 lo on the host (hi = fp8(v), lo = fp8(v - hi)), so the
# three kept product terms (hi*hi paired across contraction chunks,
# hi*lo + lo*hi packed as the two k-slices of one DoubleRow matmul)
# reconstruct the fp32 product to ~0.1% while the PE streams each
# DoubleRow output column in half a cycle.
SCALE_X = 16.0  # x ~ N(0,1): fp8 range use without hitting 240
SCALE_W = 256.0  # qkv_w ~ 0.02*N(0,1): lift lo part out of subnormals
QK_PSCALE = SCALE_X * SCALE_W  # Q,K,V carry this factor
SCALE_EFF = SCALE / (QK_PSCALE * QK_PSCALE)  # exp() logit scale

_PROG_CACHE = {}

# Filler block cost estimates (ns) for pacing the interleave.
_COST_P12_BLOCK = 850.0  # 9 fp8-DR matmuls (4.5x N=384 equiv) + drain dispatch
_COST_T_BLOCK = 2000.0  # 12 proj matmuls of N=384 + drains
_GROUP_FILL_NS = 2200.0  # target filler time between S(k) and AV(k)


def _build_program(mm_f32r=True, es_bf16=True, with_bias=True):
    import concourse.bass as bass  # noqa: F401
    import concourse.tile as tile
    from concourse import bacc, masks, mybir

    f32 = mybir.dt.float32
    f32r = mybir.dt.float32r
    bf16 = mybir.dt.bfloat16
    f8 = mybir.dt.float8e4
    DR = mybir.MatmulPerfMode.DoubleRow
    Act = mybir.ActivationFunctionType

    nc = bacc.Bacc(None, target_bir_lowering=False)
    _lp = nc.allow_low_precision(
        reason="fp8/bf16 matmul inputs, fp32 PSUM accumulation"
    )
    _lp.__enter__()

    # host-pre-split fp8 operands (see module docstring): x as
    # [p, st, chunk, (lo,hi), tok], weights as [p, chunk, (hi,lo), out-col]
    # so a (hi,lo) weight pair against a (lo,hi) x pair is exactly the two
    # cross terms of one DoubleRow matmul.
    xf8_d = nc.dram_tensor(
        "xf8", [SAMPLES, 128, 2, CCH, 2, NTOK], f8, kind="ExternalInput"
    )
    # Q,K weights m-chunk-major so each DMA wave is one contiguous run per
    # partition (strided waves cost ~2x on the serialized DMA pipe); V
    # weights column-major for the 384-wide DoubleRow moving APs.
    qkw8_d = nc.dram_tensor("qkw8", [128, MCH, CCH, 2, 128], f8, kind="ExternalInput")
    vw8_d = nc.dram_tensor("vw8", [128, CCH, 2, C], f8, kind="ExternalInput")
    projw_d = nc.dram_tensor("projwT", [128, CCH, C], bf16, kind="ExternalInput")
    bias_d = nc.dram_tensor("bias", [1, C], f32, kind="ExternalInput")
    y_d = nc.dram_tensor("y", [2 * SAMPLES, NTOK, C], f32, kind="ExternalOutput")

    with tile.TileContext(nc) as tc:
        with (
            tc.tile_pool(name="consts", bufs=1) as consts,
            tc.tile_pool(name="xtp", bufs=2) as xtp,
            tc.tile_pool(name="qktp", bufs=2) as qktp,
            tc.tile_pool(name="v1p", bufs=2) as v1p,
            tc.tile_pool(name="onp", bufs=6) as onp,
            tc.tile_pool(name="otp", bufs=1) as otp,
            tc.tile_pool(name="esap", bufs=4) as esap,
            tc.tile_pool(name="esbp", bufs=6) as esbp,
            tc.tile_pool(name="rlp", bufs=4) as rlp,
            tc.tile_pool(name="yp", bufs=3) as yp,
            tc.tile_pool(name="psp", bufs=2, space="PSUM") as psp,
            tc.tile_pool(name="pop", bufs=2, space="PSUM") as pop,
            tc.tile_pool(name="pap", bufs=2, space="PSUM") as pap,
        ):
            # ---- constant tiles ----
            qkw_sb = consts.tile([128, MCH, CCH, 2, 128], f8)
            vw_sb = consts.tile([128, CCH, 2, C], f8)
            projw_sb = consts.tile([128, CCH, C], bf16)
            bias_row = consts.tile([1, C], f32)
            bias_bc = consts.tile([128, C], f32)
            ident = consts.tile([128, 128], bf16)
            masks.make_identity(nc, ident)

            # ---- weight DMA waves, in consumption order ----
            def qk_wave(m0, m1):
                nc.sync.dma_start(out=qkw_sb[:, m0:m1], in_=qkw8_d[:, m0:m1])

            # ---- per-sample tiles (created lazily, in emission order) ----
            tiles = {}

            def xt_dma(s, st):
                xt = tiles[s][0]
                nc.sync.dma_start(out=xt[:, st], in_=xf8_d[s, :, st])

            def make_sample_tiles(s, dma=True):
                xt = xtp.tile([128, 2, CCH, 2, NTOK], f8, tag="xt", name=f"xt_{s}")
                qkt = qktp.tile([128, MCH, 2, NTOK], bf16, tag="qkt", name=f"qkt_{s}")
                v1 = v1p.tile([128, TCH, 2, H, 65], bf16, tag="v1", name=f"v1_{s}")
                # "ones" column for the softmax denominator; V carries the
                # QK_PSCALE factor, so the denominator must too.
                nc.vector.memset(v1[:, :, :, :, 64:65], QK_PSCALE)
                tiles[s] = (xt, qkt, v1)
                # one DMA per stream so P1(st=0) can start after half the
                # transfer
                if dma:
                    xt_dma(s, 0)
                    xt_dma(s, 1)

            def _dr_proj(dst, lhs_main, rhs_main, lhs_cross, rhs_cross):
                # 768-deep contraction = 3 DoubleRow mains (chunk pairs,
                # hi*hi) + 6 DoubleRow crosses (hi*lo + lo*hi per chunk),
                # one PSUM accumulation group.
                for cp in range(CCH // 2):
                    nc.tensor.matmul(
                        dst,
                        lhs_main(cp),
                        rhs_main(cp),
                        start=(cp == 0),
                        stop=False,
                        perf_mode=DR,
                    )
                for c in range(CCH):
                    nc.tensor.matmul(
                        dst,
                        lhs_cross(c),
                        rhs_cross(c),
                        start=False,
                        stop=(c == CCH - 1),
                        perf_mode=DR,
                    )

            def p1_block(s, m, st):
                xt, qkt, _ = tiles[s]
                pq = pap.tile([128, NTOK], f32, tag="pa", name=f"pq_{s}_{m}_{st}")
                _dr_proj(
                    pq,
                    lambda cp: qkw_sb[:, m, 2 * cp : 2 * cp + 2, 0, :],
                    lambda cp: xt[:, st, 2 * cp : 2 * cp + 2, 1, :],
                    lambda c: qkw_sb[:, m, c, 0:2, :],
                    lambda c: xt[:, st, c, 0:2, :],
                )
                nc.vector.tensor_copy(out=qkt[:, m, st, :], in_=pq)

            def p2_block(s, t, st, n):
                xt, _, v1 = tiles[s]
                pv = pap.tile([128, NTOK], f32, tag="pa", name=f"pv_{s}_{t}_{st}_{n}")
                ts_ = slice(t * 128, (t + 1) * 128)
                vs = slice(n * NTOK, (n + 1) * NTOK)
                _dr_proj(
                    pv,
                    lambda cp: xt[:, st, 2 * cp : 2 * cp + 2, 1, ts_],
                    lambda cp: vw_sb[:, 2 * cp : 2 * cp + 2, 0, vs],
                    lambda c: xt[:, st, c, 0:2, ts_],
                    lambda c: vw_sb[:, c, 0:2, vs],
                )
                nc.vector.tensor_copy(
                    out=v1[:, t, st, 6 * n : 6 * n + 6, 0:64],
                    in_=pv.rearrange("p (h d) -> p h d", h=6),
                )

            def t_block(s, st, t, y_dst, fine_out=False):
                # output projection + bias + DMA from the xbar-transposed
                # channel-major O tile (the transpose ran as a DMA at stream
                # end, see queue_proj)
                y_sb = yp.tile([128, C], f32, tag="y", name=f"y_{s}_{st}_{t}")
                for n2 in range(2):
                    py = pap.tile([128, NTOK], f32, tag="pa", name=f"py_{s}_{st}_{t}_{n2}")
                    for c in range(CCH):
                        nc.tensor.matmul(
                            py,
                            ot_sb[:, c, st, t * 128 : (t + 1) * 128],
                            projw_sb[:, c, n2 * NTOK : (n2 + 1) * NTOK],
                            start=(c == 0),
                            stop=(c == CCH - 1),
                        )
                    # final stream: halve the drain/DMA chunks so the last
                    # output transfer starts earlier
                    nparts = 2 if fine_out else 1
                    w = NTOK // nparts
                    for q in range(nparts):
                        lo = n2 * NTOK + q * w
                        hi = lo + w
                        if with_bias:
                            nc.vector.tensor_add(
                                y_sb[:, lo:hi], py[:, q * w : (q + 1) * w],
                                bias_bc[:, lo:hi],
                            )
                        else:
                            nc.vector.tensor_copy(
                                out=y_sb[:, lo:hi], in_=py[:, q * w : (q + 1) * w]
                            )
                        nc.sync.dma_start(
                            out=y_dst[t * 128 : (t + 1) * 128, lo:hi],
                            in_=y_sb[:, lo:hi],
                        )

            ot_sb = consts.tile([128, CCH, 2, NTOK], bf16)

            def av_norm_body(s, st, hp, esA, esB, esC, esD, v1, o_nat_qc):
                # AV, natural orientation (N=65 per matmul). The whole po bank
                # is ONE accumulation group: only the very first matmul (i=0,
                # qc=0, bank offset 0) opens it with start=True; later matmuls
                # continue with start=False (fresh addresses overwrite via
                # has_written, revisited addresses accumulate).
                po = pop.tile([128, 2, TCH, 65], f32, tag="po", name=f"po_{s}_{st}_{hp}")
                for i in range(2):
                    h = 2 * hp + i
                    nc.tensor.matmul(
                        po[:, i, 0, :],
                        esA[:, i, 0:128],
                        v1[:, 0, st, h, :],
                        start=(i == 0),
                        stop=False,
                    )
                    for qc in (1, 2):
                        q0 = qc * 128
                        dst = po[:, i, qc, :]
                        nc.tensor.matmul(
                            dst, esA[:, i, q0 : q0 + 128], v1[:, 0, st, h, :],
                            start=False, stop=False,
                        )
                        nc.tensor.matmul(
                            dst, esB[:, i, q0 - 128 : q0], v1[:, 0, 1 - st, h, :],
                            start=False, stop=False,
                        )
                        nc.tensor.matmul(
                            dst, esC[:, i, q0 - 128 : q0], v1[:, 1, st, h, :],
                            start=False, stop=False,
                        )
                        nc.tensor.matmul(
                            dst, esD[:, i, q0 - 128 : q0], v1[:, 2, st, h, :],
                            start=False, stop=(i == 1 and qc == 2),
                        )
                # normalization: batched reciprocal + scaled drain, in reverse
                # so the last DVE read of the bank covers the addresses the
                # next group's first AV matmul writes (PSUM bank-collision
                # safety via the DVE FIFO).
                rl = rlp.tile([128, 2, TCH], f32, tag="rl", name=f"rl_{s}_{st}_{hp}")
                nc.vector.reciprocal(out=rl, in_=po[:, :, :, 64])
                for i in (1, 0):
                    h = 2 * hp + i
                    for qc in (2, 1, 0):
                        nc.vector.tensor_scalar_mul(
                            o_nat_qc[qc][:, h * 64 : (h + 1) * 64],
                            po[:, i, qc, 0:64],
                            rl[:, i, qc : qc + 1],
                        )
                if s == SAMPLES - 1 and st == 1:
                    # final stream: transpose this head-pair's O columns on
                    # the PE right away instead of a post-stream xbar DMA, so
                    # the output projection can start the moment hp=5 lands.
                    ptr = pap.tile(
                        [128, TCH, 128], bf16, tag="pa", name=f"ptr_{hp}"
                    )
                    for qc in range(TCH):
                        nc.tensor.transpose(
                            ptr[:, qc, :],
                            o_nat_qc[qc][:, hp * 128 : (hp + 1) * 128],
                            ident,
                        )
                    nc.vector.tensor_copy(out=ot_sb[:, hp, 1, :], in_=ptr)

            # ---- filler queue with just-in-time forcing ----
            # Entries are [cost, key, fn, done]. Windows pop FIFO up to a
            # budget; `force(key)` emits a specific pending block immediately
            # (used to guarantee the qkt/v1 chunks a window reads were
            # produced in an earlier window). Surplus blocks ride forward
            # across sample boundaries toward the filler-starved last sample.
            fill_q = []
            reserved_q = []
            pending = {}

            def push(cost, key, fn, reserved=False):
                # reserved entries never drain eagerly: they are emitted only
                # by force() (JIT needs) or flush_all(), so they remain as
                # filler work for the starved last-stream windows.
                e = [cost, key, fn, False]
                (reserved_q if reserved else fill_q).append(e)
                if key is not None:
                    pending[key] = e

            def emit_entry(e):
                if e[3]:
                    return 0.0
                e[3] = True
                e[2]()
                if e[1] is not None:
                    pending.pop(e[1], None)
                return e[0]

            def force(key):
                e = pending.get(key)
                return emit_entry(e) if e is not None else 0.0

            def drain_fillers(budget):
                while fill_q and budget > 0.0:
                    e = fill_q[0]
                    if e[3]:
                        fill_q.pop(0)
                        continue
                    budget -= emit_entry(e)
                    fill_q.pop(0)

            def flush_all():
                for e in list(fill_q) + list(reserved_q):
                    emit_entry(e)
                fill_q.clear()
                reserved_q.clear()

            def queue_p12(s):
                last = s == SAMPLES - 1
                # the whole last sample is emitted JIT via the forcing
                # machinery so its blocks pace the filler-starved end windows
                for m in range(MCH):
                    for st in range(2):
                        push(
                            _COST_P12_BLOCK,
                            ("p1", s, m, st),
                            lambda s=s, m=m, st=st: p1_block(s, m, st),
                            reserved=last,
                        )
                for t in range(TCH):
                    for st in range(2):
                        for n in range(2):
                            push(
                                _COST_P12_BLOCK,
                                ("p2", s, t, st, n),
                                lambda s=s, t=t, st=st, n=n: p2_block(s, t, st, n),
                                reserved=last,
                            )

            held_t = []

            def queue_proj(s, st, o_nat_qc):
                # xbar DMA transposes: o_nat [tok, C] -> ot_sb [C-part, tok].
                # Issued immediately (the last o_nat drains just ran) so the
                # ~2.6us DMA latency hides before the first t_block filler.
                # The final stream transposes inline per-window instead.
                if not (s == SAMPLES - 1 and st == 1):
                    for t in range(TCH):
                        nc.sync.dma_start_transpose(
                            out=ot_sb[:, :, st, t * 128 : (t + 1) * 128],
                            in_=o_nat_qc[t][:, :],
                        )
                fine = s == SAMPLES - 1 and st == 1
                for t in range(TCH):
                    fn = lambda s=s, st=st, t=t, f=fine: t_block(
                        s, st, t, y_d[2 * s + st], fine_out=f
                    )
                    if s == SAMPLES - 1 and st == 0:
                        # reserve for the filler-starved last windows
                        held_t.append(fn)
                    else:
                        push(_COST_T_BLOCK, ("t", s, st, t), fn)

            def s_needs(s, st, hp):
                return (
                    ("p1", s, hp, st),
                    ("p1", s, 6 + hp, st),
                    ("p1", s, 6 + hp, 1 - st),
                )

            def av_needs(s, st, hp):
                n = hp // 3
                return (
                    ("p2", s, 0, st, n),
                    ("p2", s, 1, st, n),
                    ("p2", s, 2, st, n),
                    ("p2", s, 0, 1 - st, n),
                )

            # ---- sample 0 projections run inline ----
            # st-outer so P1(st=0) starts as soon as the st=0 half of xt and
            # the first weight wave land; weight waves interleave in
            # consumption order.
            make_sample_tiles(0, dma=False)
            qk_wave(0, 2)
            xt_dma(0, 0)
            qk_wave(2, 4)
            qk_wave(4, 6)
            qk_wave(6, 9)
            qk_wave(9, 12)
            xt_dma(0, 1)
            nc.sync.dma_start(out=vw_sb, in_=vw8_d[:, :, :, :])
            nc.sync.dma_start(out=projw_sb, in_=projw_d[:, :, :])
            nc.sync.dma_start(out=bias_row, in_=bias_d[:, :])
            nc.gpsimd.partition_broadcast(bias_bc, bias_row)
            # Warm-up: keep the PE continuously busy through the input-DMA
            # wait so the 3us p-state ramp runs down on throwaway matmuls
            # instead of the first real projection blocks. A memset source
            # is ready ~1us before the identity tile.
            warm = pap.tile([128, NTOK], f32, tag="pa", name="warm")
            wz = consts.tile([128, 128], bf16)
            nc.vector.memset(wz, 0.0)
            for _ in range(35):
                nc.tensor.matmul(
                    warm[:, 0:128], wz, wz, start=True, stop=True
                )
            for st in range(2):
                for m in range(MCH):
                    p1_block(0, m, st)
            make_sample_tiles(1)
            for t in range(TCH):
                for st in range(2):
                    for n in range(2):
                        p2_block(0, t, st, n)

            # ---- main loop ----
            for s in range(SAMPLES):
                _, qkt, v1 = tiles[s]
                if s + 1 < SAMPLES:
                    queue_p12(s + 1)
                windows = [(st, hp) for st in range(2) for hp in range(6)]
                for st in range(2):
                    o_nat_qc = [
                        onp.tile([128, C], bf16, tag="on", name=f"on_{s}_{st}_{qc}")
                        for qc in range(TCH)
                    ]
                    for hp in range(6):
                        # backstop: anything this window reads must exist now
                        forced = 0.0
                        for k in s_needs(s, st, hp) + av_needs(s, st, hp):
                            forced += force(k)
                        # ---- scores (S.T = K Q.T), bf16, transposed layout ----
                        # Every matmul writes a whole single-bank tile at
                        # offset 0 (matmuls writing at a non-zero PSUM bank
                        # offset fault on hardware). 8 tiles rotate through 4
                        # banks; the exp of each tile is emitted right after
                        # its matmul so the bank frees quickly.
                        esA = esap.tile([128, 2, NTOK], bf16, tag="esa", name=f"esA_{s}_{st}_{hp}")
                        esB = esbp.tile([128, 2, 256], bf16, tag="esb", name=f"esB_{s}_{st}_{hp}")
                        esC = esbp.tile([128, 2, 256], bf16, tag="esb", name=f"esC_{s}_{st}_{hp}")
                        esD = esbp.tile([128, 2, 256], bf16, tag="esb", name=f"esD_{s}_{st}_{hp}")
                        # Both i-halves of a letter land in one [128, 2, 512]
                        # PSUM tile (i=1 starts exactly at the next bank's
                        # offset 0) so ONE exp covers both — halves the ACT
                        # instruction count, which is the latency bottleneck
                        # in filler-starved stretches.
                        for letter, lkf, rqf, nq, es in (
                            ("A", lambda i, kT, kTo: kT[:, 0:MT],
                             lambda i, qT, qTs: qT, NTOK, esA),
                            ("B", lambda i, kT, kTo: kTo[:, 0:MT],
                             lambda i, qT, qTs: qTs, 256, esB),
                            ("C", lambda i, kT, kTo: kT[:, MT : MT + 128],
                             lambda i, qT, qTs: qTs, 256, esC),
                            ("D", lambda i, kT, kTo: kT[:, MT + 128 : MT + 256],
                             lambda i, qT, qTs: qTs, 256, esD),
                        ):
                            ps = psp.tile(
                                [128, 2, 512], f32, tag="ps",
                                name=f"ps{letter}_{s}_{st}_{hp}",
                            )
                            for i in range(2):
                                ro = 64 * i
                                qT = qkt[ro : ro + 64, hp, st, :]
                                qTs = qkt[ro : ro + 64, hp, st, MT:]
                                kT = qkt[ro : ro + 64, 6 + hp, st, :]
                                kTo = qkt[ro : ro + 64, 6 + hp, 1 - st, :]
                                nc.tensor.matmul(
                                    ps[:, i, 0:nq], lkf(i, kT, kTo),
                                    rqf(i, qT, qTs), start=True, stop=True,
                                )
                            nc.scalar.activation(
                                es[:, :, :], ps[:, :, 0:nq], Act.Exp,
                                scale=SCALE_EFF,
                            )

                        # ---- fillers: hide exp latency under projection work ----
                        # lookahead: produce the NEXT window's inputs here so
                        # the next window's scores/AV never wait on a fresh
                        # qkt/v1 write
                        wi = windows.index((st, hp))
                        for la in (1, 2):
                            if wi + la < len(windows):
                                nst, nhp = windows[wi + la]
                                for k in s_needs(s, nst, nhp) + av_needs(s, nst, nhp):
                                    forced += force(k)
                        if s + 1 < SAMPLES:
                            # pre-produce the next sample's first window late in
                            # this sample
                            if (st, hp) == (1, 4):
                                for k in av_needs(s + 1, 0, 0):
                                    forced += force(k)
                            if (st, hp) == (1, 5):
                                for k in s_needs(s + 1, 0, 0) + av_needs(s + 1, 0, 0):
                                    forced += force(k)
                        # deadline for deferred output-projection blocks: the
                        # o_nat buffers they read are overwritten one stream
                        # later, so (s-1, st1) must run during (s, st0) and
                        # (s, st0) during (s, st1)
                        if hp >= 3:
                            if st == 1 and s == SAMPLES - 1:
                                if held_t:
                                    held_t.pop(0)()
                                    forced += _COST_T_BLOCK
                            else:
                                tk = (
                                    ("t", s - 1, 1, hp - 3)
                                    if st == 0
                                    else ("t", s, 0, hp - 3)
                                )
                                forced += force(tk)
                        drain_fillers(max(0.0, _GROUP_FILL_NS - forced))

                        av_norm_body(s, st, hp, esA, esB, esC, esD, v1, o_nat_qc)


                    # ---- end of stream: defer this stream's projections ----
                    queue_proj(s, st, o_nat_qc)
                    if s == 0 and st == 0:
                        # xt for sample 1 was created before p2; issue s+1 early
                        pass

                # ---- end of sample ----
                if s + 1 < SAMPLES:
                    if s + 2 < SAMPLES:
                        make_sample_tiles(s + 2)
                else:
                    for fn in held_t:
                        fn()
                    held_t.clear()
                    flush_all()

    _lp.__exit__(None, None, None)
    nc.compile()
    return nc


def _get_program(mm_f32r=True, es_bf16=True, with_bias=True):
    key = (bool(with_bias),)
    if key not in _PROG_CACHE:
        _PROG_CACHE[key] = _build_program(with_bias=bool(with_bias))
    return _PROG_CACHE[key]


def _split_f8(v):
    """Split float32 v into e4m3 hi + e4m3 lo with hi + lo ~= v to ~0.1%."""
    import ml_dtypes

    F8 = ml_dtypes.float8_e4m3
    hi = np.clip(v, -240.0, 240.0).astype(F8)
    lo = np.clip(v - hi.astype(np.float32), -240.0, 240.0).astype(F8)
    return hi, lo


def _prep_in_maps(x_v, x_i, qkv_w, proj_w, proj_b):
    import ml_dtypes

    # weights: [C, 3C] channel-major -> [p, chunk, (hi,lo), col], scaled
    wT = np.asarray(qkv_w, np.float32).T.reshape(CCH, 128, 3 * C)
    wT = np.ascontiguousarray(wT.transpose(1, 0, 2)) * SCALE_W
    whi, wlo = _split_f8(wT)
    qkvw8 = np.stack([whi, wlo], axis=2)  # [p, c, hl, 3C]
    qkw8 = np.ascontiguousarray(
        qkvw8[:, :, :, : 2 * C]
        .reshape(128, CCH, 2, MCH, 128)
        .transpose(0, 3, 1, 2, 4)
    )
    vw8 = np.ascontiguousarray(qkvw8[:, :, :, 2 * C :])
    projwT = np.ascontiguousarray(
        np.asarray(proj_w, np.float32).T.reshape(CCH, 128, C).transpose(1, 0, 2)
    ).astype(ml_dtypes.bfloat16)
    bias = np.ascontiguousarray(np.asarray(proj_b).astype(np.float32).reshape(1, C))
    in_maps = []
    for core in range(N_CORES):
        sl = slice(core * SAMPLES, (core + 1) * SAMPLES)
        # x: [S, tok, C] -> [S, p, st, chunk, (lo,hi), tok], scaled
        xs = np.empty((SAMPLES, 128, 2, CCH, NTOK), np.float32)
        xs[:, :, 0] = (
            np.asarray(x_v[sl]).transpose(0, 2, 1).reshape(SAMPLES, CCH, 128, NTOK)
        ).transpose(0, 2, 1, 3)
        xs[:, :, 1] = (
            np.asarray(x_i[sl]).transpose(0, 2, 1).reshape(SAMPLES, CCH, 128, NTOK)
        ).transpose(0, 2, 1, 3)
        xs *= SCALE_X
        xhi, xlo = _split_f8(xs)
        xf8 = np.ascontiguousarray(np.stack([xlo, xhi], axis=4))
        in_maps.append(
            {
                "xf8": xf8,
                "qkw8": qkw8,
                "vw8": vw8,
                "projwT": projwT,
                "bias": bias,
            }
        )
    return in_maps


def kernel(x_v, x_i, qkv_w, proj_w, proj_b, t_h, t_w, s_h, s_w, num_heads):
    from concourse.bass_utils import run_bass_kernel_spmd

    x_v = np.asarray(x_v, np.float32)
    x_i = np.asarray(x_i, np.float32)
    nc = _get_program(with_bias=bool(np.any(np.asarray(proj_b))))
    in_maps = _prep_in_maps(x_v, x_i, qkv_w, proj_w, proj_b)
    res = run_bass_kernel_spmd(nc, in_maps, list(range(N_CORES)))
    out_v = np.empty((B, NTOK, C), np.float32)
    out_i = np.empty((B, NTOK, C), np.float32)
    for core in range(N_CORES):
        y = res.results[core]["y"]
        sl = slice(core * SAMPLES, (core + 1) * SAMPLES)
        out_v[sl] = y[0::2]
        out_i[sl] = y[1::2]
    return out_v, out_i



# revision 48
# speedup vs baseline: 1.0585x; 1.0585x over previous
"""Trainium2 Bass kernel for the two-template sparse cross-modal attention module.

Sharding: data-parallel over batch B=32 across 8 NeuronCores (4 samples/core).
Each sample carries two modality streams (v, i) that must be co-resident
because search tokens attend to the template keys of BOTH modalities.

Per-core program (per sample s, streams st in {v, i}), all matmuls bf16 with
fp32 PSUM accumulation (inputs are cast to bf16 by the gpsimd DMAs, which
also halves input HBM traffic):
  1. QK^T projection in transposed layout (QKT[1536, 384] = qkv_w[0:1536] @
     x.T, stored bf16); V projection in natural layout with a ones column per
     head ([tok, 65]) so the AV matmul also accumulates the softmax
     denominator.
  2. Scores transposed (S.T[k, q] = K Q.T, contract Dh=64), exp on the ACT
     engine (no max-subtraction; logits are O(1)). Each score matmul writes a
     whole single-bank PSUM tile at offset 0: a start=True accumulation-group
     open at a non-zero PSUM bank offset faults on real hardware.
  3. AV in natural orientation: out[q, 65] = es.T @ V1 with es (the exp'ed
     score tile) as the stationary operand — each matmul streams only 65
     columns instead of 256/384, which is what the PE cost scales with. The
     six accumulation chains of a head-pair share one PSUM bank as ONE group
     (only the offset-0 chain opens it). Normalization = DVE batched
     reciprocal of the ones column + per-head scaled PSUM-drain copies into
     the natural-layout O tile (bf16), emitted in reverse so the bank is
     fully drained before the next group's first AV write (PSUM banks are
     single-ported; a PE write concurrent with a DVE read anywhere in the
     bank is fatal).
  4. O is transposed back to channel-major via PE transpose instructions
     (bf16 identity), then the output projection runs from the transposed
     tile; the bias is added by the DVE during the PSUM drain (no K=1 bias
     matmuls).

Scheduling: projection matmul blocks are held in a keyed filler queue and
interleaved between each head-pair's score and AV matmuls so the ACT engine's
exp latency (the attention-phase bottleneck) hides under PE work. Each window
force-emits just-in-time the P1/P2 blocks the next window's scores/AV read
and the deferred output-projection blocks whose o_nat buffers are about to be
recycled; surplus blocks ride across sample boundaries toward the
filler-starved last sample. Input DMAs are issued as batched multi-chunk
waves in exact consumption order, so the first projection matmul starts ~6us
into the program.

Cost-model numbers (TimelineSim, per core): 288us total, PE busy 265us
(vs 412us / 290us for the previous kernel).
"""

import numpy as np

for _p in ("/opt/trn_rl_repo", "/root/.axon_site/_ro/trn_rl_repo"):
    import os
    import sys

    if os.path.isdir(_p) and _p not in sys.path:
        sys.path.append(_p)

B = 32
N_CORES = 8
SAMPLES = 4  # per core
C = 768
NTOK = 384
H = 12
DH = 64
MT = 128  # template tokens
CCH = C // 128  # 6 contraction chunks
MCH = 12  # QK row chunks (1536/128)
TCH = NTOK // 128  # 3 token chunks
SCALE = DH ** (-0.5)
# fp8 double-row projection path: inputs are scaled then split into
# e4m3 hi + e4m3 lo on the host (hi = fp8(v), lo = fp8(v - hi)), so the
# three kept product terms (hi*hi paired across contraction chunks,
# hi*lo + lo*hi packed as the two k-slices of one DoubleRow matmul)
# reconstruct the fp32 product to ~0.1% while the PE streams each
# DoubleRow output column in half a cycle.
SCALE_X = 16.0  # x ~ N(0,1): fp8 range use without hitting 240
SCALE_W = 256.0  # qkv_w ~ 0.02*N(0,1): lift lo part out of subnormals
QK_PSCALE = SCALE_X * SCALE_W  # Q,K,V carry this factor
SCALE_EFF = SCALE / (QK_PSCALE * QK_PSCALE)  # exp() logit scale

_PROG_CACHE = {}

# Filler block cost estimates (ns) for pacing the interleave.
_COST_P12_BLOCK = 850.0  # 9 fp8-DR matmuls (4.5x N=384 equiv) + drain dispatch
_COST_T_BLOCK = 2000.0  # 12 proj matmuls of N=384 + drains
_GROUP_FILL_NS = 2200.0  # target filler time between S(k) and AV(k)


def _build_program(mm_f32r=True, es_bf16=True, with_bias=True):
    import concourse.bass as bass  # noqa: F401
    import concourse.tile as tile
    from concourse import bacc, masks, mybir

    f32 = mybir.dt.float32
    f32r = mybir.dt.float32r
    bf16 = mybir.dt.bfloat16
    f8 = mybir.dt.float8e4
    DR = mybir.MatmulPerfMode.DoubleRow
    Act = mybir.ActivationFunctionType

    nc = bacc.Bacc(None, target_bir_lowering=False)
    _lp = nc.allow_low_precision(
        reason="fp8/bf16 matmul inputs, fp32 PSUM accumulation"
    )
    _lp.__enter__()

    # host-pre-split fp8 operands (see module docstring): x as
    # [p, st, chunk, (lo,hi), tok], weights as [p, chunk, (hi,lo), out-col]
    # so a (hi,lo) weight pair against a (lo,hi) x pair is exactly the two
    # cross terms of one DoubleRow matmul.
    xf8_d = nc.dram_tensor(
        "xf8", [SAMPLES, 128, 2, CCH, 2, NTOK], f8, kind="ExternalInput"
    )
    # Q,K weights m-chunk-major so each DMA wave is one contiguous run per
    # partition (strided waves cost ~2x on the serialized DMA pipe); V
    # weights column-major for the 384-wide DoubleRow moving APs.
    qkw8_d = nc.dram_tensor("qkw8", [128, MCH, CCH, 2, 128], f8, kind="ExternalInput")
    vw8_d = nc.dram_tensor("vw8", [128, CCH, 2, C], f8, kind="ExternalInput")
    projw_d = nc.dram_tensor("projwT", [128, CCH, C], bf16, kind="ExternalInput")
    bias_d = nc.dram_tensor("bias", [1, C], f32, kind="ExternalInput")
    y_d = nc.dram_tensor("y", [2 * SAMPLES, NTOK, C], f32, kind="ExternalOutput")

    with tile.TileContext(nc) as tc:
        with (
            tc.tile_pool(name="consts", bufs=1) as consts,
            tc.tile_pool(name="xtp", bufs=2) as xtp,
            tc.tile_pool(name="qktp", bufs=2) as qktp,
            tc.tile_pool(name="v1p", bufs=2) as v1p,
            tc.tile_pool(name="onp", bufs=6) as onp,
            tc.tile_pool(name="otp", bufs=1) as otp,
            tc.tile_pool(name="esap", bufs=4) as esap,
            tc.tile_pool(name="esbp", bufs=6) as esbp,
            tc.tile_pool(name="rlp", bufs=4) as rlp,
            tc.tile_pool(name="yp", bufs=3) as yp,
            tc.tile_pool(name="psp", bufs=2, space="PSUM") as psp,
            tc.tile_pool(name="pop", bufs=2, space="PSUM") as pop,
            tc.tile_pool(name="pap", bufs=2, space="PSUM") as pap,
        ):
            # ---- constant tiles ----
            qkw_sb = consts.tile([128, MCH, CCH, 2, 128], f8)
            vw_sb = consts.tile([128, CCH, 2, C], f8)
            projw_sb = consts.tile([128, CCH, C], bf16)
            bias_row = consts.tile([1, C], f32)
            bias_bc = consts.tile([128, C], f32)
            ident = consts.tile([128, 128], bf16)
            masks.make_identity(nc, ident)

            # ---- weight DMA waves, in consumption order ----
            def qk_wave(m0, m1):
                nc.sync.dma_start(out=qkw_sb[:, m0:m1], in_=qkw8_d[:, m0:m1])

            # ---- per-sample tiles (created lazily, in emission order) ----
            tiles = {}

            def xt_dma(s, st):
                xt = tiles[s][0]
                nc.sync.dma_start(out=xt[:, st], in_=xf8_d[s, :, st])

            def make_sample_tiles(s, dma=True):
                xt = xtp.tile([128, 2, CCH, 2, NTOK], f8, tag="xt", name=f"xt_{s}")
                qkt = qktp.tile([128, MCH, 2, NTOK], bf16, tag="qkt", name=f"qkt_{s}")
                v1 = v1p.tile([128, TCH, 2, H, 65], bf16, tag="v1", name=f"v1_{s}")
                # "ones" column for the softmax denominator; V carries the
                # QK_PSCALE factor, so the denominator must too.
                nc.vector.memset(v1[:, :, :, :, 64:65], QK_PSCALE)
                tiles[s] = (xt, qkt, v1)
                # one DMA per stream so P1(st=0) can start after half the
                # transfer
                if dma:
                    xt_dma(s, 0)
                    xt_dma(s, 1)

            def _dr_proj(dst, lhs_main, rhs_main, lhs_cross, rhs_cross):
                # 768-deep contraction = 3 DoubleRow mains (chunk pairs,
                # hi*hi) + 6 DoubleRow crosses (hi*lo + lo*hi per chunk),
                # one PSUM accumulation group.
                for cp in range(CCH // 2):
                    nc.tensor.matmul(
                        dst,
                        lhs_main(cp),
                        rhs_main(cp),
                        start=(cp == 0),
                        stop=False,
                        perf_mode=DR,
                    )
                for c in range(CCH):
                    nc.tensor.matmul(
                        dst,
                        lhs_cross(c),
                        rhs_cross(c),
                        start=False,
                        stop=(c == CCH - 1),
                        perf_mode=DR,
                    )

            def p1_block(s, m, st):
                xt, qkt, _ = tiles[s]
                pq = pap.tile([128, NTOK], f32, tag="pa", name=f"pq_{s}_{m}_{st}")
                _dr_proj(
                    pq,
                    lambda cp: qkw_sb[:, m, 2 * cp : 2 * cp + 2, 0, :],
                    lambda cp: xt[:, st, 2 * cp : 2 * cp + 2, 1, :],
                    lambda c: qkw_sb[:, m, c, 0:2, :],
                    lambda c: xt[:, st, c, 0:2, :],
                )
                nc.vector.tensor_copy(out=qkt[:, m, st, :], in_=pq)

            def p2_block(s, t, st, n):
                xt, _, v1 = tiles[s]
                pv = pap.tile([128, NTOK], f32, tag="pa", name=f"pv_{s}_{t}_{st}_{n}")
                ts_ = slice(t * 128, (t + 1) * 128)
                vs = slice(n * NTOK, (n + 1) * NTOK)
                _dr_proj(
                    pv,
                    lambda cp: xt[:, st, 2 * cp : 2 * cp + 2, 1, ts_],
                    lambda cp: vw_sb[:, 2 * cp : 2 * cp + 2, 0, vs],
                    lambda c: xt[:, st, c, 0:2, ts_],
                    lambda c: vw_sb[:, c, 0:2, vs],
                )
                nc.vector.tensor_copy(
                    out=v1[:, t, st, 6 * n : 6 * n + 6, 0:64],
                    in_=pv.rearrange("p (h d) -> p h d", h=6),
                )

            def t_block(s, st, t, y_dst, fine_out=False):
                # output projection + bias + DMA from the xbar-transposed
                # channel-major O tile (the transpose ran as a DMA at stream
                # end, see queue_proj)
                y_sb = yp.tile([128, C], f32, tag="y", name=f"y_{s}_{st}_{t}")
                for n2 in range(2):
                    py = pap.tile([128, NTOK], f32, tag="pa", name=f"py_{s}_{st}_{t}_{n2}")
                    for c in range(CCH):
                        nc.tensor.matmul(
                            py,
                            ot_sb[:, c, st, t * 128 : (t + 1) * 128],
                            projw_sb[:, c, n2 * NTOK : (n2 + 1) * NTOK],
                            start=(c == 0),
                            stop=(c == CCH - 1),
                        )
                    lo = n2 * NTOK
                    hi = lo + NTOK
                    beng = nc.vector
                    if with_bias:
                        beng.tensor_add(y_sb[:, lo:hi], py, bias_bc[:, lo:hi])
                    else:
                        beng.tensor_copy(out=y_sb[:, lo:hi], in_=py)
                # one DMA per t_block (not per n2 half): the single-slot HWDGE
                # serializes issue at 625ns each, which dominates the tail
                nc.sync.dma_start(
                    out=y_dst[t * 128 : (t + 1) * 128, :], in_=y_sb[:, :]
                )

            ot_sb = consts.tile([128, CCH, 2, NTOK], bf16)

            def av_norm_body(s, st, hp, esA, esB, esC, esD, v1, o_nat_qc):
                # AV, natural orientation (N=65 per matmul). The whole po bank
                # is ONE accumulation group: only the very first matmul (i=0,
                # qc=0, bank offset 0) opens it with start=True; later matmuls
                # continue with start=False (fresh addresses overwrite via
                # has_written, revisited addresses accumulate).
                po = pop.tile([128, 2, TCH, 65], f32, tag="po", name=f"po_{s}_{st}_{hp}")
                for i in range(2):
                    h = 2 * hp + i
                    nc.tensor.matmul(
                        po[:, i, 0, :],
                        esA[:, i, 0:128],
                        v1[:, 0, st, h, :],
                        start=(i == 0),
                        stop=False,
                    )
                    for qc in (1, 2):
                        q0 = qc * 128
                        dst = po[:, i, qc, :]
                        nc.tensor.matmul(
                            dst, esA[:, i, q0 : q0 + 128], v1[:, 0, st, h, :],
                            start=False, stop=False,
                        )
                        nc.tensor.matmul(
                            dst, esB[:, i, q0 - 128 : q0], v1[:, 0, 1 - st, h, :],
                            start=False, stop=False,
                        )
                        nc.tensor.matmul(
                            dst, esC[:, i, q0 - 128 : q0], v1[:, 1, st, h, :],
                            start=False, stop=False,
                        )
                        nc.tensor.matmul(
                            dst, esD[:, i, q0 - 128 : q0], v1[:, 2, st, h, :],
                            start=False, stop=(i == 1 and qc == 2),
                        )
                # normalization: batched reciprocal + scaled drain, in reverse
                # so the engine's last read of the bank covers the addresses
                # the next group's first AV matmul writes (PSUM bank-collision
                # safety via the engine FIFO).
                drain_eng = nc.vector
                rl = rlp.tile([128, 2, TCH], f32, tag="rl", name=f"rl_{s}_{st}_{hp}")
                nc.vector.reciprocal(out=rl, in_=po[:, :, :, 64])
                for i in (1, 0):
                    h = 2 * hp + i
                    for qc in (2, 1, 0):
                        drain_eng.tensor_scalar_mul(
                            o_nat_qc[qc][:, h * 64 : (h + 1) * 64],
                            po[:, i, qc, 0:64],
                            rl[:, i, qc : qc + 1],
                        )
                if s == SAMPLES - 1 and st == 1:
                    # final stream: transpose this head-pair's O columns on
                    # the PE right away instead of a post-stream xbar DMA, so
                    # the output projection can start the moment hp=5 lands.
                    ptr = pap.tile(
                        [128, TCH, 128], bf16, tag="pa", name=f"ptr_{hp}"
                    )
                    for qc in range(TCH):
                        nc.tensor.transpose(
                            ptr[:, qc, :],
                            o_nat_qc[qc][:, hp * 128 : (hp + 1) * 128],
                            ident,
                        )
                    nc.vector.tensor_copy(out=ot_sb[:, hp, 1, :], in_=ptr)

            # ---- filler queue with just-in-time forcing ----
            # Entries are [cost, key, fn, done]. Windows pop FIFO up to a
            # budget; `force(key)` emits a specific pending block immediately
            # (used to guarantee the qkt/v1 chunks a window reads were
            # produced in an earlier window). Surplus blocks ride forward
            # across sample boundaries toward the filler-starved last sample.
            fill_q = []
            reserved_q = []
            pending = {}

            def push(cost, key, fn, reserved=False):
                # reserved entries never drain eagerly: they are emitted only
                # by force() (JIT needs) or flush_all(), so they remain as
                # filler work for the starved last-stream windows.
                e = [cost, key, fn, False]
                (reserved_q if reserved else fill_q).append(e)
                if key is not None:
                    pending[key] = e

            def emit_entry(e):
                if e[3]:
                    return 0.0
                e[3] = True
                e[2]()
                if e[1] is not None:
                    pending.pop(e[1], None)
                return e[0]

            def force(key):
                e = pending.get(key)
                return emit_entry(e) if e is not None else 0.0

            def drain_fillers(budget, allow_reserved=False):
                while budget > 0.0:
                    q = fill_q if fill_q else (reserved_q if allow_reserved else None)
                    if not q:
                        return
                    e = q.pop(0)
                    if e[3]:
                        continue
                    budget -= emit_entry(e)

            def flush_all():
                for e in list(fill_q) + list(reserved_q):
                    emit_entry(e)
                fill_q.clear()
                reserved_q.clear()

            def queue_p12(s):
                # The last sample's st1 blocks (Q, K and V) are demand-paced:
                # emitted only when window lookahead forces them, so they
                # remain as exp-hiding filler for the end-of-program windows
                # instead of being consumed early by sample 2's windows.
                last = s == SAMPLES - 1
                for m in range(MCH):
                    for st in range(2):
                        push(
                            _COST_P12_BLOCK,
                            ("p1", s, m, st),
                            lambda s=s, m=m, st=st: p1_block(s, m, st),
                            reserved=last and st == 1,
                        )
                for t in range(TCH):
                    for st in range(2):
                        for n in range(2):
                            push(
                                _COST_P12_BLOCK,
                                ("p2", s, t, st, n),
                                lambda s=s, t=t, st=st, n=n: p2_block(s, t, st, n),
                                reserved=last and st == 1,
                            )

            held_t = []

            def queue_proj(s, st, o_nat_qc):
                # xbar DMA transposes: o_nat [tok, C] -> ot_sb [C-part, tok].
                # Issued immediately (the last o_nat drains just ran) so the
                # ~2.6us DMA latency hides before the first t_block filler.
                # The final stream transposes inline per-window instead.
                if not (s == SAMPLES - 1 and st == 1):
                    for t in range(TCH):
                        nc.sync.dma_start_transpose(
                            out=ot_sb[:, :, st, t * 128 : (t + 1) * 128],
                            in_=o_nat_qc[t][:, :],
                        )
                fine = False  # AB-test
                for t in range(TCH):
                    fn = lambda s=s, st=st, t=t, f=fine: t_block(
                        s, st, t, y_d[2 * s + st], fine_out=f
                    )
                    if s == SAMPLES - 1 and st == 0:
                        # reserve for the filler-starved last windows
                        held_t.append(fn)
                    else:
                        push(_COST_T_BLOCK, ("t", s, st, t), fn)

            def s_needs(s, st, hp):
                return (
                    ("p1", s, hp, st),
                    ("p1", s, 6 + hp, st),
                    ("p1", s, 6 + hp, 1 - st),
                )

            def av_needs(s, st, hp):
                n = hp // 3
                return (
                    ("p2", s, 0, st, n),
                    ("p2", s, 1, st, n),
                    ("p2", s, 2, st, n),
                    ("p2", s, 0, 1 - st, n),
                )

            # ---- sample 0 projections run inline ----
            # st-outer so P1(st=0) starts as soon as the st=0 half of xt and
            # the first weight wave land; weight waves interleave in
            # consumption order.
            make_sample_tiles(0, dma=False)
            qk_wave(0, 2)
            xt_dma(0, 0)
            qk_wave(2, 4)
            qk_wave(4, 6)
            qk_wave(6, 9)
            qk_wave(9, 12)
            xt_dma(0, 1)
            nc.sync.dma_start(out=vw_sb, in_=vw8_d[:, :, :, :])
            nc.sync.dma_start(out=projw_sb, in_=projw_d[:, :, :])
            nc.sync.dma_start(out=bias_row, in_=bias_d[:, :])
            nc.gpsimd.partition_broadcast(bias_bc, bias_row)
            # Warm-up: keep the PE continuously busy through the input-DMA
            # wait so the 3us p-state ramp runs down on throwaway matmuls
            # instead of the first real projection blocks. A memset source
            # is ready ~1us before the identity tile.
            warm = pap.tile([128, NTOK], f32, tag="pa", name="warm")
            wz = consts.tile([128, 128], bf16)
            nc.vector.memset(wz, 0.0)
            for _ in range(35):
                nc.tensor.matmul(
                    warm[:, 0:128], wz, wz, start=True, stop=True
                )
            for st in range(2):
                for m in range(MCH):
                    p1_block(0, m, st)
            make_sample_tiles(1)
            for t in range(TCH):
                for st in range(2):
                    for n in range(2):
                        p2_block(0, t, st, n)

            # ---- main loop ----
            # Windows are software-pipelined: window k's scores (and their
            # exps) are emitted BEFORE window k-1's AV, so the ACT engine's
            # four-exp chain for k overlaps the PE's AV/drain work for k-1.
            # Without this, ACT idles ~0.9us per window whenever fillers run
            # dry (the whole last sample), since AV(k) gates on exp(k, D).
            pend = [None]  # deferred AV: (s, v1, st, hp, esA..D, o_nat_qc)

            def flush_av():
                if pend[0] is None:
                    return
                ps_, pv1, pst, php, eA, eB, eC, eD, onat = pend[0]
                pend[0] = None
                av_norm_body(ps_, pst, php, eA, eB, eC, eD, pv1, onat)
                if php == 5:
                    # stream complete: defer its output projections
                    queue_proj(ps_, pst, onat)

            for s in range(SAMPLES):
                _, qkt, v1 = tiles[s]
                if s + 1 < SAMPLES:
                    queue_p12(s + 1)
                windows = [(st, hp) for st in range(2) for hp in range(6)]
                for st in range(2):
                    o_nat_qc = [
                        onp.tile([128, C], bf16, tag="on", name=f"on_{s}_{st}_{qc}")
                        for qc in range(TCH)
                    ]
                    for hp in range(6):
                        # backstop: anything this window reads must exist now
                        forced = 0.0
                        for k in s_needs(s, st, hp) + av_needs(s, st, hp):
                            forced += force(k)
                        # ---- scores (S.T = K Q.T), bf16, transposed layout ----
                        # Every matmul writes a whole single-bank tile at
                        # offset 0 (matmuls writing at a non-zero PSUM bank
                        # offset fault on hardware). 8 tiles rotate through 4
                        # banks; the exp of each tile is emitted right after
                        # its matmul so the bank frees quickly.
                        esA = esap.tile([128, 2, NTOK], bf16, tag="esa", name=f"esA_{s}_{st}_{hp}")
                        esB = esbp.tile([128, 2, 256], bf16, tag="esb", name=f"esB_{s}_{st}_{hp}")
                        esC = esbp.tile([128, 2, 256], bf16, tag="esb", name=f"esC_{s}_{st}_{hp}")
                        esD = esbp.tile([128, 2, 256], bf16, tag="esb", name=f"esD_{s}_{st}_{hp}")
                        # Both i-halves of a letter land in one [128, 2, 512]
                        # PSUM tile (i=1 starts exactly at the next bank's
                        # offset 0) so ONE exp covers both — halves the ACT
                        # instruction count, which is the latency bottleneck
                        # in filler-starved stretches.
                        for letter, lkf, rqf, nq, es in (
                            ("A", lambda i, kT, kTo: kT[:, 0:MT],
                             lambda i, qT, qTs: qT, NTOK, esA),
                            ("B", lambda i, kT, kTo: kTo[:, 0:MT],
                             lambda i, qT, qTs: qTs, 256, esB),
                            ("C", lambda i, kT, kTo: kT[:, MT : MT + 128],
                             lambda i, qT, qTs: qTs, 256, esC),
                            ("D", lambda i, kT, kTo: kT[:, MT + 128 : MT + 256],
                             lambda i, qT, qTs: qTs, 256, esD),
                        ):
                            ps = psp.tile(
                                [128, 2, 512], f32, tag="ps",
                                name=f"ps{letter}_{s}_{st}_{hp}",
                            )
                            for i in range(2):
                                ro = 64 * i
                                qT = qkt[ro : ro + 64, hp, st, :]
                                qTs = qkt[ro : ro + 64, hp, st, MT:]
                                kT = qkt[ro : ro + 64, 6 + hp, st, :]
                                kTo = qkt[ro : ro + 64, 6 + hp, 1 - st, :]
                                nc.tensor.matmul(
                                    ps[:, i, 0:nq], lkf(i, kT, kTo),
                                    rqf(i, qT, qTs), start=True, stop=True,
                                )
                            nc.scalar.activation(
                                es[:, :, :], ps[:, :, 0:nq], Act.Exp,
                                scale=SCALE_EFF,
                            )

                        # ---- fillers: hide exp latency under projection work ----
                        # lookahead: produce the NEXT window's inputs here so
                        # the next window's scores/AV never wait on a fresh
                        # qkt/v1 write
                        wi = windows.index((st, hp))
                        for la in (1, 2):
                            if wi + la < len(windows):
                                nst, nhp = windows[wi + la]
                                for k in s_needs(s, nst, nhp) + av_needs(s, nst, nhp):
                                    forced += force(k)
                        if s + 1 < SAMPLES:
                            # pre-produce the next sample's first window late in
                            # this sample
                            if (st, hp) == (1, 4):
                                for k in av_needs(s + 1, 0, 0):
                                    forced += force(k)
                            if (st, hp) == (1, 5):
                                for k in s_needs(s + 1, 0, 0) + av_needs(s + 1, 0, 0):
                                    forced += force(k)
                        # deadline for deferred output-projection blocks: the
                        # o_nat buffers they read are overwritten one stream
                        # later, so (s-1, st1) must run during (s, st0) and
                        # (s, st0) during (s, st1). Forced in the FIRST three
                        # windows: those are the filler-starved ones (the
                        # later windows pull the next stream's JIT blocks).
                        if hp >= 3:
                            if st == 1 and s == SAMPLES - 1:
                                if held_t:
                                    held_t.pop(0)()
                                    forced += _COST_T_BLOCK
                            else:
                                tk = (
                                    ("t", s - 1, 1, hp - 3)
                                    if st == 0
                                    else ("t", s, 0, hp - 3)
                                )
                                forced += force(tk)
                        drain_fillers(
                            max(0.0, _GROUP_FILL_NS - forced),
                            allow_reserved=s == SAMPLES - 1,
                        )

                        flush_av()
                        pend[0] = (s, v1, st, hp, esA, esB, esC, esD, o_nat_qc)

                # ---- end of sample (pend carries into the next sample) ----
                if s + 1 < SAMPLES:
                    if s + 2 < SAMPLES:
                        make_sample_tiles(s + 2)
                else:
                    flush_av()
                    for fn in held_t:
                        fn()
                    held_t.clear()
                    flush_all()

    _lp.__exit__(None, None, None)
    nc.compile()
    return nc


def _get_program(mm_f32r=True, es_bf16=True, with_bias=True):
    key = (bool(with_bias),)
    if key not in _PROG_CACHE:
        _PROG_CACHE[key] = _build_program(with_bias=bool(with_bias))
    return _PROG_CACHE[key]


def _split_f8(v):
    """Split float32 v into e4m3 hi + e4m3 lo with hi + lo ~= v to ~0.1%."""
    import ml_dtypes

    F8 = ml_dtypes.float8_e4m3
    hi = np.clip(v, -240.0, 240.0).astype(F8)
    lo = np.clip(v - hi.astype(np.float32), -240.0, 240.0).astype(F8)
    return hi, lo


def _prep_in_maps(x_v, x_i, qkv_w, proj_w, proj_b):
    import ml_dtypes

    # weights: [C, 3C] channel-major -> [p, chunk, (hi,lo), col], scaled
    wT = np.asarray(qkv_w, np.float32).T.reshape(CCH, 128, 3 * C)
    wT = np.ascontiguousarray(wT.transpose(1, 0, 2)) * SCALE_W
    whi, wlo = _split_f8(wT)
    qkvw8 = np.stack([whi, wlo], axis=2)  # [p, c, hl, 3C]
    qkw8 = np.ascontiguousarray(
        qkvw8[:, :, :, : 2 * C]
        .reshape(128, CCH, 2, MCH, 128)
        .transpose(0, 3, 1, 2, 4)
    )
    vw8 = np.ascontiguousarray(qkvw8[:, :, :, 2 * C :])
    projwT = np.ascontiguousarray(
        np.asarray(proj_w, np.float32).T.reshape(CCH, 128, C).transpose(1, 0, 2)
    ).astype(ml_dtypes.bfloat16)
    bias = np.ascontiguousarray(np.asarray(proj_b).astype(np.float32).reshape(1, C))
    in_maps = []
    for core in range(N_CORES):
        sl = slice(core * SAMPLES, (core + 1) * SAMPLES)
        # x: [S, tok, C] -> [S, p, st, chunk, (lo,hi), tok], scaled
        xs = np.empty((SAMPLES, 128, 2, CCH, NTOK), np.float32)
        xs[:, :, 0] = (
            np.asarray(x_v[sl]).transpose(0, 2, 1).reshape(SAMPLES, CCH, 128, NTOK)
        ).transpose(0, 2, 1, 3)
        xs[:, :, 1] = (
            np.asarray(x_i[sl]).transpose(0, 2, 1).reshape(SAMPLES, CCH, 128, NTOK)
        ).transpose(0, 2, 1, 3)
        xs *= SCALE_X
        xhi, xlo = _split_f8(xs)
        xf8 = np.ascontiguousarray(np.stack([xlo, xhi], axis=4))
        in_maps.append(
            {
                "xf8": xf8,
                "qkw8": qkw8,
                "vw8": vw8,
                "projwT": projwT,
                "bias": bias,
            }
        )
    return in_maps


def kernel(x_v, x_i, qkv_w, proj_w, proj_b, t_h, t_w, s_h, s_w, num_heads):
    from concourse.bass_utils import run_bass_kernel_spmd

    x_v = np.asarray(x_v, np.float32)
    x_i = np.asarray(x_i, np.float32)
    nc = _get_program(with_bias=bool(np.any(np.asarray(proj_b))))
    in_maps = _prep_in_maps(x_v, x_i, qkv_w, proj_w, proj_b)
    res = run_bass_kernel_spmd(nc, in_maps, list(range(N_CORES)))
    out_v = np.empty((B, NTOK, C), np.float32)
    out_i = np.empty((B, NTOK, C), np.float32)
    for core in range(N_CORES):
        y = res.results[core]["y"]
        sl = slice(core * SAMPLES, (core + 1) * SAMPLES)
        out_v[sl] = y[0::2]
        out_i[sl] = y[1::2]
    return out_v, out_i

